# revision 51
# baseline (speedup 1.0000x reference)
"""Trainium2 Bass kernel for nn_ReconstructionHead (dense_mlp).

Computes, for x[B=256, T=513, D=512] (CLS token at t=512 dropped):
    h   = x[:, :512] @ W1.T + b1          # [256, 512, 512]
    h   = LayerNorm(h) * gamma + beta     # over last dim
    h   = relu(h)
    out[b, t] = h[b, t] @ Wout[t] + bout[t]   # [256, 512]

Sharding: data-parallel over batch across 8 NeuronCores (32 batches/core).
Weights are replicated. All input reshaping/transposition happens on the
host (numpy); the device sees clean strided layouts.

Per-core device program (fast path, gamma==1 / beta==0), all matmul and
stage-2 data in bf16 (rel err ~3.5e-3 vs the 2e-2 gate; PSUM/stats fp32):
  - x is pre-transposed on host to xt[p, g, dc, m] with d = dc*128+p on
    SBUF partitions (16384 rows -> 128 tiles of 128 rows, 32 groups).
  - PE: per tile, a K=128 matmul (ones[128,128] @ b1rep/128) seeds PSUM
    with b1 - K=128 so its LDWEIGHTS pipelines like the mains (a K=1
    rank-1 seed costs two ~100ns boundary bubbles) - then 4 accumulating
    bf16 128x128x512 matmuls produce h1 = x @ W1.T + b1 (negated).
  - DVE bn_stats/bn_aggr give mean/var per row straight from PSUM.
  - 1-in-4 tiles (rotating position, exactly one per group so DVE never
    bursts over its group budget): a runtime-registered custom DVE op
    relu(-P + mean) * wout with accum_out does relu+head in ONE pass.
  - Other tiles: ACT relu from PSUM (per-partition bias = bn mean;
    rstd multiply deferred: relu((h1-mu)*rstd) = rstd*relu(h1-mu)),
    GPSIMD multiply by wout, ACT copy-accumulate row-sum. The ACT
    copy-accums are software-pipelined one tile late so the in-order
    ACT queue never parks on a not-yet-ready GPSIMD result (the ACT
    queue must keep releasing PSUM banks via the relus, or the PE
    stalls on bank recycle).
  - Per-group epilogue (deferred one group for the same reason):
    rstd = sqrt(reciprocal(var)) with recip FIRST on DVE so DVE never
    waits on ACT; out_col = s * rstd + bout into a [128,128] SBUF tile.
  - One 64KB output DMA at the end; the [p, c] -> m transpose is free on
    the host.
"""

import os
import sys

import numpy as np

for _p in ("/root/.axon_site/_ro/trn_rl_repo", "/opt/trn_rl_repo"):
    if os.path.isdir(_p) and _p not in sys.path:
        sys.path.append(_p)

B = 256
T = 513
D = 512          # d_in == d_out
NCORES = 8
BL = B // NCORES          # 32 batches per core
M = BL * D                # 16384 rows per core
NT = M // 128             # 128 tiles per core
NG = NT // 4              # 32 groups (one group = 512 rows = one batch)
EPS = 1e-5

_programs = {}


def _matmul_dtype():
    """'bf16' (default), 'f32r', or 'f32' via KERNEL_DTYPE."""
    d = os.environ.get("KERNEL_DTYPE", "bf16")
    if os.environ.get("KERNEL_FP32_STRICT") == "1":
        return "f32"
    if d == "bf16":
        try:
            import ml_dtypes  # noqa: F401
        except ImportError:
            return "f32r"
    return d


def _register_fused_op():
    """Register a custom DVE op: out = relu(in0*s0 + s1) * in1 with
    accum_out = row-sum. One DVE pass straight from PSUM replaces the
    ACT relu + multiply + reduce chain of the per-column head stage.
    Self-pins the uop sha (computed and checked in the same process)."""
    if os.environ.get("KERNEL_NO_FUSED") == "1":
        return None
    try:
        from operator import add as _add

        from concourse import dve_ops
        from concourse.dve_spec import (
            C0, C1, Spec, Src0, Src1, Zero, relu, lower, _has_src1,
        )
        from concourse.dve_uop import DveOpSpec

        name = "RELU_AFF_MUL_REDUCE"
        for o in dve_ops.OPS:
            if o.name == name:
                return o

        def _ref(in0, in1, s0, s1, imm2):
            a = np.nan_to_num(
                in0.astype(np.float32) * s0 + s1,
                nan=0.0, posinf=np.inf, neginf=-np.inf,
            )
            b = (np.maximum(a, 0) * in1).astype(np.float32)
            return b, b.reshape(b.shape[0], -1).sum(axis=-1, keepdims=True)

        spec = Spec(
            body=relu(Src0 * C0 + C1) * Src1,
            accum=_add,
            accum_init=Zero,
            reference=_ref,
        )
        row = max(dve_ops._SUB_OPCODE_FOR_NAME.values()) + 1
        if row >= 0x20:
            return None
        shas = {
            ver: DveOpSpec(
                name=name, opcode=row, uops=lower(spec, ver=ver),
                rd1_en=_has_src1(spec),
            ).sha(ver)
            for ver in ("v3", "v4")
        }
        op = dve_ops.DveOp(name, spec, subdim=False, uops_sha=shas)
        dve_ops.OPS.append(op)
        dve_ops.CUSTOM_DVE_SPECS[name] = spec
        dve_ops._SUB_OPCODE_FOR_NAME[name] = row
        return op
    except Exception:
        return None


def _build_program(apply_gamma_beta: bool):
    import concourse.bacc as bacc
    import concourse.tile as tile
    from concourse import mybir

    f32 = mybir.dt.float32
    dt_mode = _matmul_dtype()
    # bf16 matmuls stream at the same 1 cycle/row as float32r but get
    # FWL on LDWEIGHTS (4x faster weight loads that hide under the
    # previous matmul) and halve DMA + stage-2 vector-engine traffic.
    if dt_mode == "bf16":
        f32m = mybir.dt.bfloat16
    elif dt_mode == "f32r":
        f32m = mybir.dt.float32r
    else:
        f32m = f32
    # vector-side dtype for relu output / stage-2 operands
    vd = mybir.dt.bfloat16 if dt_mode == "bf16" else f32
    Alu = mybir.AluOpType
    Act = mybir.ActivationFunctionType
    fused = _register_fused_op()

    nc = bacc.Bacc()
    xt = nc.dram_tensor("xt", [128, NG, 4, 512], f32m, kind="ExternalInput")
    w1t = nc.dram_tensor("w1t", [128, 4, D], f32m, kind="ExternalInput")
    # b1/128 replicated over 128 partitions: the bias seed is a K=128
    # matmul (ones.T @ b1rep) so its LDWEIGHTS pipelines exactly like the
    # main matmuls (a K=1 rank-1 seed costs two ~100ns boundary bubbles)
    b1 = nc.dram_tensor("b1", [128, D], f32m, kind="ExternalInput")
    wout = nc.dram_tensor("wout", [128, 4, D], vd, kind="ExternalInput")
    bout = nc.dram_tensor("bout", [128, 4], f32, kind="ExternalInput")
    if apply_gamma_beta:
        gammab = nc.dram_tensor("gammab", [128, D], f32, kind="ExternalInput")
        betab = nc.dram_tensor("betab", [128, D], f32, kind="ExternalInput")
    # out[p, c] = output for row m = c*128 + p (transposed on host)
    out = nc.dram_tensor("out", [128, 128], f32, kind="ExternalOutput")

    with tile.TileContext(nc) as tc:
        with (
            tc.tile_pool(name="singles", bufs=1) as singles,
            tc.tile_pool(name="xg", bufs=4) as xpool,
            tc.tile_pool(name="u", bufs=8) as upool,
            tc.tile_pool(name="junk", bufs=4) as jpool,
            tc.tile_pool(name="stats", bufs=12) as spool,
            tc.tile_pool(name="grp", bufs=4) as gpool,
            tc.tile_pool(name="psum", bufs=8, space="PSUM") as psum_pool,
        ):
            # ---- static tiles ----
            # ordered so the first matmul's dependencies land first
            b1_sb = singles.tile([128, D], f32m)
            nc.sync.dma_start(b1_sb, b1[:, :])
            w1t_sb = singles.tile([128, 4, D], f32m)
            nc.sync.dma_start(w1t_sb, w1t[:, :, :])

            def load_group(g):
                xg = xpool.tile([128, 4, 512], f32m, tag="xg")
                nc.sync.dma_start(xg, xt[:, g, :, :])
                return xg

            xg_next = load_group(0)

            wout_sb = singles.tile([128, 4, D], vd)
            nc.sync.dma_start(wout_sb, wout[:, :, :])
            bout_sb = singles.tile([128, 4], f32)
            nc.sync.dma_start(bout_sb, bout[:, :])
            if f32m is mybir.dt.float32r:
                ones_sb = singles.tile([128, 128], f32)
                nc.vector.memset(ones_sb, 1.0)
                ones_mm = ones_sb.bitcast(f32m)
            else:
                ones_mm = singles.tile([128, 128], f32m)
                nc.vector.memset(ones_mm, 1.0)
            eps_sb = singles.tile([128, 1], f32)
            nc.vector.memset(eps_sb, EPS)
            ocol = singles.tile([128, 128], f32)  # per-tile output columns
            if apply_gamma_beta:
                gamma_sb = singles.tile([128, D], f32)
                nc.sync.dma_start(gamma_sb, gammab[:, :])
                beta_sb = singles.tile([128, D], f32)
                nc.sync.dma_start(beta_sb, betab[:, :])

            # Software pipelining (fast path): ACT copy-accums and the
            # per-group epilogue are emitted one tile / one group late, so
            # the in-order ACT/GPSIMD/DVE queues never park on an
            # instruction whose cross-engine input isn't ready yet.
            pend_copy = []   # (junk, sg_slice, g, kind) awaiting reduce
            pend_epi = None  # (sg, mvg, g) awaiting epilogue
            nf_count = [0]   # non-fused tile counter (ACT/DVE reduce mix)

            def flush_copies(upto_g):
                keep = []
                for jt, sg_ap, gg, kind in pend_copy:
                    if gg <= upto_g:
                        if kind == "dve":
                            # occasional DVE row-sum relieves the ACT
                            # accumulate+drain (the busiest engine)
                            nc.vector.tensor_reduce(
                                out=sg_ap, in_=jt,
                                axis=mybir.AxisListType.X, op=Alu.add,
                            )
                        else:
                            nc.scalar.activation(
                                jt, jt, Act.Copy, bias=0.0, scale=1.0,
                                accum_out=sg_ap,
                            )
                    else:
                        keep.append((jt, sg_ap, gg, kind))
                pend_copy[:] = keep

            def emit_epilogue(sg_t, mvg_t, g_idx):
                # rstd = sqrt(1/var): reciprocal FIRST on DVE (depends
                # only on DVE's own bn_aggr), sqrt on ACT; a
                # sqrt->reciprocal order would stall the in-order DVE
                # queue on ACT for ~2.8us per group. eps=1e-5 is dropped:
                # var is O(0.3) here so it shifts rstd by ~2e-5 relative,
                # far below the bf16 noise floor.
                rg = gpool.tile([128, 4], f32, tag="rg")
                nc.vector.reciprocal(rg, mvg_t[:, :, 1])
                rstd = gpool.tile([128, 4], f32, tag="rstd")
                nc.scalar.activation(rstd, rg, Act.Sqrt)
                tmp = gpool.tile([128, 4], f32, tag="tmp")
                nc.gpsimd.tensor_mul(tmp, sg_t, rstd)
                nc.gpsimd.tensor_add(
                    ocol[:, g_idx * 4:(g_idx + 1) * 4], tmp, bout_sb
                )

            for g in range(NG):
                xg = xg_next
                if g + 1 < NG:
                    xg_next = load_group(g + 1)

                mvg = gpool.tile([128, 4, 2], f32)   # (mean, var) per tile
                sg = gpool.tile([128, 4], f32)       # stage-2 raw sums

                for i in range(4):
                    c = g * 4 + i
                    P = psum_pool.tile([128, 512], f32)
                    # seed PSUM with b1 (rank-1 matmul), then accumulate x@W1T
                    nc.tensor.matmul(P, ones_mm, b1_sb, start=True, stop=False)
                    for dc in range(4):
                        nc.tensor.matmul(
                            P,
                            xg[:, dc, i * 128:(i + 1) * 128],
                            w1t_sb[:, dc, :],
                            start=False,
                            stop=(dc == 3),
                        )

                    st6 = spool.tile([128, 6], f32)
                    nc.vector.bn_stats(st6, P)
                    nc.vector.bn_aggr(mvg[:, i, :], st6)

                    if not apply_gamma_beta and fused is not None and i == (g % 4):
                        # fused path: one custom DVE op straight from PSUM
                        # computes junk = relu(-P + mean) * wout and
                        # accumulates the row sum - replaces the ACT relu,
                        # the multiply, and the reduce for this tile.
                        junk = jpool.tile([128, 512], vd)
                        nc.vector._custom_dve(
                            fused,
                            out=junk,
                            in0=P,
                            in1=wout_sb[:, i, :],
                            s0=-1.0,
                            s1=mvg[:, i, 0:1],
                            accum_out=sg[:, i:i + 1],
                        )
                        if i == 0 and pend_epi is not None:
                            flush_copies(g - 1)
                            emit_epilogue(*pend_epi)
                            pend_epi = None
                        continue

                    if not apply_gamma_beta:
                        # Host negated W1T/b1, so P holds -h1 and bn_stats'
                        # mean is -mu: u = relu(-1*P + mean) = relu(h1 - mu).
                        # rstd multiplication is deferred to the epilogue.
                        u = upool.tile([128, 512], vd)
                        nc.scalar.activation(
                            u, P, Act.Relu, bias=mvg[:, i, 0:1], scale=-1.0
                        )
                        # older tiles' deferred ACT copies: their GPSIMD
                        # inputs are ready by now, so they don't park the
                        # ACT queue (which must keep releasing PSUM banks)
                        flush_copies(NG)
                        junk = jpool.tile([128, 512], vd)
                        nc.gpsimd.tensor_mul(junk, u, wout_sb[:, i, :])
                        # every 7th non-fused tile reduces on DVE instead
                        # of ACT copy-accum: ACT is the pacing engine
                        # (~169us busy) and DVE has ~24us of headroom
                        nf_count[0] += 1
                        kind = "dve" if nf_count[0] % 7 == 3 else "act"
                        pend_copy.append((junk, sg[:, i:i + 1], g, kind))
                        if i == 0 and pend_epi is not None:
                            flush_copies(g - 1)
                            emit_epilogue(*pend_epi)
                            pend_epi = None
                        continue
                    else:
                        # full path: n = (h1 - mu) * rstd ; z = n*gamma + beta
                        sd = spool.tile([128, 1], f32, tag="sd")
                        nc.scalar.activation(
                            sd, mvg[:, i, 1:2], Act.Sqrt, bias=eps_sb, scale=1.0
                        )
                        rr = spool.tile([128, 1], f32, tag="rr")
                        nc.vector.reciprocal(rr, sd)
                        n_sb = upool.tile([128, 512], f32, tag="n")
                        nc.vector.tensor_scalar(
                            out=n_sb,
                            in0=P,
                            scalar1=mvg[:, i, 0:1],
                            scalar2=rr,
                            op0=Alu.subtract,
                            op1=Alu.mult,
                        )
                        v_sb = upool.tile([128, 512], f32, tag="v")
                        nc.gpsimd.tensor_mul(v_sb, n_sb, gamma_sb)
                        z_sb = upool.tile([128, 512], f32, tag="z")
                        nc.vector.tensor_add(z_sb, v_sb, beta_sb)
                        u = upool.tile([128, 512], vd)
                        nc.scalar.activation(u, z_sb, Act.Relu)

                    # stage-2: s = sum_e u * Wout[t-block i]
                    junk = jpool.tile([128, 512], vd)
                    if fused is None and (c % 2) == 0:
                        # fused multiply+row-sum on DVE. Only used when the
                        # custom op is unavailable: mixing the native
                        # stt accumulator with the custom-DVE accumulator
                        # in one program wedged the DVE in probing.
                        nc.vector.scalar_tensor_tensor(
                            out=junk,
                            in0=u,
                            scalar=0.0,
                            in1=wout_sb[:, i, :],
                            op0=Alu.add,
                            op1=Alu.mult,
                            accum_out=sg[:, i:i + 1],
                        )
                    else:
                        # GPSIMD multiply, ACT row-sum via accumulate
                        nc.gpsimd.tensor_mul(junk, u, wout_sb[:, i, :])
                        nc.scalar.activation(
                            junk, junk, Act.Copy, bias=0.0, scale=1.0,
                            accum_out=sg[:, i:i + 1],
                        )

                # ---- per-group epilogue (deferred one group, fast path) ----
                if not apply_gamma_beta:
                    pend_epi = (sg, mvg, g)
                else:
                    nc.vector.tensor_add(
                        ocol[:, g * 4:(g + 1) * 4], sg, bout_sb
                    )

            flush_copies(NG)
            if pend_epi is not None:
                emit_epilogue(*pend_epi)

            # single 64KB output DMA; the [p, c] -> m = c*128 + p transpose
            # happens on the host (free)
            nc.sync.dma_start(out[:, :], ocol)

    nc.finalize()
    return nc


def _get_program(apply_gamma_beta: bool):
    key = bool(apply_gamma_beta)
    if key not in _programs:
        _programs[key] = _build_program(key)
    return _programs[key]


def kernel(**inputs) -> np.ndarray:
    x = np.asarray(inputs["x"], dtype=np.float32)
    W1 = np.asarray(inputs["W1"], dtype=np.float32)
    b1 = np.asarray(inputs["b1"], dtype=np.float32)
    gamma = np.asarray(inputs["gamma"], dtype=np.float32)
    beta = np.asarray(inputs["beta"], dtype=np.float32)
    Wout = np.asarray(inputs["Wout"], dtype=np.float32)
    bout = np.asarray(inputs["bout"], dtype=np.float32)

    assert x.shape == (B, T, D), x.shape

    fast = bool(np.all(gamma == 1.0) and np.all(beta == 0.0))
    nc = _get_program(apply_gamma_beta=not fast)

    # ---- host-side packing (free at device time) ----
    # W1 is [e, d]; device wants W1T chunks [p, dc, e] with d = dc*128 + p.
    # Fast path: negate W1T/b1 so PSUM holds -h1 and the bn_stats mean can be
    # used directly as the relu bias (relu(-P + mean) == relu(h1 - mu)).
    dt_mode = _matmul_dtype()
    if dt_mode == "bf16":
        import ml_dtypes

        mm_np = ml_dtypes.bfloat16
    else:
        mm_np = np.float32
    sgn = np.float32(-1.0 if fast else 1.0)
    w1t_np = np.ascontiguousarray(
        (sgn * W1.T).reshape(4, 128, D).transpose(1, 0, 2).astype(mm_np)
    )
    wout_np = np.ascontiguousarray(
        Wout.reshape(4, 128, D).transpose(1, 0, 2).astype(mm_np)
    )
    bout_np = np.ascontiguousarray(bout.reshape(4, 128).T)
    b1_np = np.ascontiguousarray(
        np.broadcast_to((sgn / 128.0) * b1, (128, D)).astype(mm_np)
    )

    shared = {"w1t": w1t_np, "b1": b1_np, "wout": wout_np, "bout": bout_np}
    if not fast:
        shared["gammab"] = np.ascontiguousarray(
            np.broadcast_to(gamma, (128, D))
        )
        shared["betab"] = np.ascontiguousarray(
            np.broadcast_to(beta, (128, D))
        )

    # drop CLS -> [256, 512, 512]; cast before the big permute so the
    # transpose moves half the bytes
    xs = np.asarray(x[:, : T - 1, :], dtype=mm_np)
    in_maps = []
    for c in range(NCORES):
        src = xs[c * BL:(c + 1) * BL].reshape(M, D)
        # [m, d] -> [p, g, dc, mm] with d = dc*128 + p, m = g*512 + mm
        xt_c = np.ascontiguousarray(
            src.reshape(NG, 512, 4, 128).transpose(3, 0, 2, 1)
        )
        in_maps.append({"xt": xt_c, **shared})

    from concourse import bass_utils

    trace = os.environ.get("KERNEL_TRACE") == "1"
    res = bass_utils.run_bass_kernel_spmd(
        nc, in_maps, core_ids=list(range(NCORES)), trace=trace
    )
    if trace:
        if res.exec_time_ns is not None:
            print(f"HW exec time: {res.exec_time_ns} ns")
            print(f"mean exec time: {res.mean_exec_time_ns} ns "
                  f"(slowest core {res.max_exec_time_core_id})")
        if res.instructions_and_trace is not None:
            print("trace:", res.instructions_and_trace[1])
        if res.profile_json is not None:
            print("profile json:", res.profile_json)

    out_full = np.empty((B, D), dtype=np.float32)
    for c, r in enumerate(res.results):
        # device out[p, tc] holds row m = tc*128 + p
        out_full[c * BL:(c + 1) * BL] = (
            np.ascontiguousarray(r["out"].T).reshape(BL, D)
        )
    return out_full

